# revision 28
# baseline (speedup 1.0000x reference)
"""Trainium2 Bass kernel for the ComplexSSM problem.

Math (per batch b, channel r):
    lam = -5*sigmoid(lambda_raw); mag = exp(lam); a = mag*exp(i*omega)
    x[t] = W_proj @ u[t]                       (real)
    h[t] = a*h[t-1] + x[t],  h[-1] = h0        (complex, diagonal)
    y[t] = concat(h_r[t], h_i[t]) + W_res @ u[t]
    out  = layernorm(y) * gamma + beta
    finals = h[T-1]

Polar decomposition of the scan (the key trick):
    h[t] = exp(i*omega*(t+1)) * g[t]
    g[t] = mag*g[t-1] + exp(-i*omega*(t+1))*x[t],   g[-1] = h0
so with C[t]=cos(omega*(t+1)), S[t]=sin(omega*(t+1)) (host fp64 tables):
    g_r = scan(mag, x*C, op1=add),  g_i = scan(mag, x*S, op1=subtract)
    h_r = C*g_r - S*g_i,            h_i = C*g_i + S*g_r
Each scan is a real first-order recurrence with constant per-partition
coefficient -> native tensor_tensor_scan on the DVE.

Device layout per core (one batch per NeuronCore, 8 cores):
    - time processed in chunks of CH=512, software-pipelined so the PE
      stream of chunk i's front end (u transpose + x matmuls) comes before
      chunk i-1's back end (residual matmuls + h transposes + layernorm),
      keeping the PE dense while the scan chain runs on DVE/GPSIMD.
    - channel dim (R=512) packed as 4 partition-tiles side by side in the
      free dim: "big tiles" [128, 4*CH] so elementwise work runs in few
      large instructions.
    - rotation products are computed in place to save SBUF.
"""

import sys

if "/opt/trn_rl_repo" not in sys.path:
    sys.path.insert(0, "/opt/trn_rl_repo")

import numpy as np

import concourse.bacc as bacc
import concourse.mybir as mybir
import concourse.tile as tile
from concourse.bass_utils import run_bass_kernel_spmd

# Problem constants (hardcoded per the contract)
B, T, D, R = 8, 4096, 1024, 512
CH = 512          # time chunk (scan segment length)
NCH = T // CH     # 8 chunks
RT = R // 128     # 4 channel tiles
DT = D // 128     # 8 contraction tiles
SUB = CH // 128   # 4 psum output subtiles per chunk
BIG = RT * CH     # packed free size of channel-major big tiles
LN_EPS = 1e-5

F32 = mybir.dt.float32
F32R = mybir.dt.float32r
AX = mybir.AluOpType
AF = mybir.ActivationFunctionType

_CACHE = {}


def build_program(mm_dtype=F32, apply_gamma_beta=False):
    nc = bacc.Bacc("TRN2", target_bir_lowering=False, debug=False, num_devices=B)

    u_b = nc.dram_tensor("u_b", [T, D], F32, kind="ExternalInput").ap()
    h0r = nc.dram_tensor("h0r", [R, 1], F32, kind="ExternalInput").ap()
    h0i = nc.dram_tensor("h0i", [R, 1], F32, kind="ExternalInput").ap()
    wpt = nc.dram_tensor("wpt", [D, R], F32, kind="ExternalInput").ap()      # W_proj.T
    wrt = nc.dram_tensor("wrt", [D, 2 * R], F32, kind="ExternalInput").ap()  # W_res.T
    # channel-packed rotation tables: [128, RT, T]
    ctab = nc.dram_tensor("ctab", [128, RT, T], F32, kind="ExternalInput").ap()
    stab = nc.dram_tensor("stab", [128, RT, T], F32, kind="ExternalInput").ap()
    magb = nc.dram_tensor("magb", [128, BIG], F32, kind="ExternalInput").ap()
    idn = nc.dram_tensor("idn", [128, 128], F32, kind="ExternalInput").ap()
    epsb = nc.dram_tensor("epsb", [128, 1], F32, kind="ExternalInput").ap()
    gmb = nc.dram_tensor("gmb", [128, 2 * R], F32, kind="ExternalInput").ap()
    btb = nc.dram_tensor("btb", [128, 2 * R], F32, kind="ExternalInput").ap()

    out_b = nc.dram_tensor("out_b", [T, 2 * R], F32, kind="ExternalOutput").ap()
    fin_r = nc.dram_tensor("fin_r", [R, 1], F32, kind="ExternalOutput").ap()
    fin_i = nc.dram_tensor("fin_i", [R, 1], F32, kind="ExternalOutput").ap()

    WDT = mm_dtype

    with tile.TileContext(nc) as tc:
        with tc.tile_pool(name="const", bufs=1) as cp, \
             tc.tile_pool(name="work", bufs=1) as wp, \
             tc.tile_pool(name="small", bufs=2) as sp, \
             tc.tile_pool(name="pt", bufs=2, space="PSUM") as ptp, \
             tc.tile_pool(name="px", bufs=2, space="PSUM") as pxp, \
             tc.tile_pool(name="py", bufs=2, space="PSUM") as pyp:

            # ---- static loads ----
            ident = cp.tile([128, 128], F32, name="ident", tag="ident")
            nc.sync.dma_start(out=ident, in_=idn)
            epst = cp.tile([128, 1], F32, name="epst", tag="epst")
            nc.sync.dma_start(out=epst, in_=epsb)
            wpt_t = []
            wrt_t = []
            for d in range(DT):
                w1 = cp.tile([128, R], WDT, name=f"wpt{d}", tag=f"wpt{d}")
                w2 = cp.tile([128, 2 * R], WDT, name=f"wrt{d}", tag=f"wrt{d}")
                nc.sync.dma_start(out=w1,
                                  in_=wpt[d * 128:(d + 1) * 128, :].bitcast(WDT))
                nc.sync.dma_start(out=w2,
                                  in_=wrt[d * 128:(d + 1) * 128, :].bitcast(WDT))
                wpt_t.append(w1)
                wrt_t.append(w2)
            mag_t = cp.tile([128, BIG], F32, name="mag_t", tag="mag_t")
            nc.sync.dma_start(out=mag_t, in_=magb)
            h0r_t = []
            h0i_t = []
            for g in range(RT):
                hr0 = cp.tile([128, 1], F32, name=f"h0r{g}", tag=f"h0r{g}")
                nc.sync.dma_start(out=hr0, in_=h0r[g * 128:(g + 1) * 128, :])
                h0r_t.append(hr0)
                hi0 = cp.tile([128, 1], F32, name=f"h0i{g}", tag=f"h0i{g}")
                nc.sync.dma_start(out=hi0, in_=h0i[g * 128:(g + 1) * 128, :])
                h0i_t.append(hi0)
            if apply_gamma_beta:
                gam = cp.tile([128, 2 * R], F32, name="gam", tag="gam")
                nc.sync.dma_start(out=gam, in_=gmb)
                bet = cp.tile([128, 2 * R], F32, name="bet", tag="bet")
                nc.sync.dma_start(out=bet, in_=btb)

            carry_r = None
            carry_i = None
            state = {}

            def front(c):
                """u load/transpose, x matmuls, rotations + scans for chunk c."""
                nonlocal carry_r, carry_i
                uraw = []
                for s in range(SUB):
                    ur = wp.tile([128, D], F32, name=f"uraw{s}", tag=f"uraw{s}",
                                 bufs=1)
                    r0 = c * CH + s * 128
                    nc.gpsimd.dma_start(out=ur, in_=u_b[r0:r0 + 128, :])
                    uraw.append(ur)

                uT = []
                for d in range(DT):
                    pt = ptp.tile([128, CH], F32, name=f"pt{d}", tag="pt")
                    for s in range(SUB):
                        nc.tensor.matmul(
                            pt[:, s * 128:(s + 1) * 128],
                            lhsT=uraw[s][:, d * 128:(d + 1) * 128],
                            rhs=ident, is_transpose=True,
                            start=True, stop=True)
                    ut = wp.tile([128, CH], WDT, name=f"uT{d}", tag=f"uT{d}",
                                 bufs=2)
                    nc.scalar.copy(ut, pt)
                    uT.append(ut)

                ct = wp.tile([128, BIG], F32, name="ct", tag="ct", bufs=2)
                nc.scalar.dma_start(
                    out=ct.rearrange("p (g t) -> p g t", g=RT),
                    in_=ctab[:, :, c * CH:(c + 1) * CH])
                st = wp.tile([128, BIG], F32, name="st", tag="st", bufs=2)
                nc.scalar.dma_start(
                    out=st.rearrange("p (g t) -> p g t", g=RT),
                    in_=stab[:, :, c * CH:(c + 1) * CH])

                # x projection -> xsb (via ScalarE copy off PSUM)
                xsb = wp.tile([128, BIG], F32, name="xsb", tag="xsb", bufs=2)
                for g in range(RT):
                    px = pxp.tile([128, CH], F32, name=f"px{g}", tag="px")
                    for d in range(DT):
                        nc.tensor.matmul(
                            px,
                            lhsT=wpt_t[d][:, g * 128:(g + 1) * 128],
                            rhs=uT[d],
                            start=(d == 0), stop=(d == DT - 1))
                    nc.scalar.copy(xsb[:, g * CH:(g + 1) * CH], px)

                # pre-rotation (DVE): xtr = x*C, xti = x*S
                xtr = wp.tile([128, BIG], F32, name="xtr", tag="xa", bufs=1)
                nc.vector.tensor_tensor(xtr, xsb, ct, op=AX.mult)
                xti = wp.tile([128, BIG], F32, name="xti", tag="xb", bufs=1)
                nc.vector.tensor_tensor(xti, xsb, st, op=AX.mult)

                # scans (DVE) per channel-tile segment; carries per segment
                gr = wp.tile([128, BIG], F32, name="gr", tag="gA", bufs=2)
                gi = wp.tile([128, BIG], F32, name="gi", tag="gB", bufs=2)
                ncar_r = []
                ncar_i = []
                for g in range(RT):
                    sl = slice(g * CH, (g + 1) * CH)
                    init_r = h0r_t[g] if c == 0 else carry_r[g]
                    nc.vector.tensor_tensor_scan(
                        gr[:, sl], mag_t[:, sl], xtr[:, sl], init_r,
                        op0=AX.mult, op1=AX.add)
                    init_i = h0i_t[g] if c == 0 else carry_i[g]
                    nc.vector.tensor_tensor_scan(
                        gi[:, sl], mag_t[:, sl], xti[:, sl], init_i,
                        op0=AX.mult, op1=AX.subtract)
                    ncr = sp.tile([128, 1], F32, name=f"ncr{g}", tag=f"ncr{g}",
                                  bufs=2)
                    nc.vector.tensor_copy(ncr, gr[:, (g + 1) * CH - 1:(g + 1) * CH])
                    ncar_r.append(ncr)
                    nci = sp.tile([128, 1], F32, name=f"nci{g}", tag=f"nci{g}",
                                  bufs=2)
                    nc.vector.tensor_copy(nci, gi[:, (g + 1) * CH - 1:(g + 1) * CH])
                    ncar_i.append(nci)
                carry_r, carry_i = ncar_r, ncar_i

                # post-rotation (all DVE; GPSIMD is avoided entirely since it
                # contends for the DVE SBUF port):
                #   hr = C*gr - S*gi ; hi = C*gi + S*gr
                ta = wp.tile([128, BIG], F32, name="ta", tag="xa", bufs=1)
                nc.vector.tensor_tensor(ta, ct, gr, op=AX.mult)
                tb = wp.tile([128, BIG], F32, name="tb", tag="xb", bufs=1)
                nc.vector.tensor_tensor(tb, st, gi, op=AX.mult)
                hr = wp.tile([128, BIG], F32, name="hr", tag="gA", bufs=2)
                nc.vector.tensor_tensor(hr, ta, tb, op=AX.subtract)
                tc_ = wp.tile([128, BIG], F32, name="tc_", tag="xa", bufs=1)
                nc.vector.tensor_tensor(tc_, ct, gi, op=AX.mult)
                td = wp.tile([128, BIG], F32, name="td", tag="xb", bufs=1)
                nc.vector.tensor_tensor(td, st, gr, op=AX.mult)
                hi = wp.tile([128, BIG], F32, name="hi", tag="gB", bufs=2)
                nc.vector.tensor_tensor(hi, tc_, td, op=AX.add)
                state[c] = (uT, hr, hi)

            def back(c):
                """residual matmuls + h transposes + layernorm for chunk c."""
                uT, hr, hi = state.pop(c)
                for s in range(SUB):
                    py = pyp.tile([128, 2 * R], F32, name="py", tag="py")
                    for d in range(DT):
                        nc.tensor.matmul(
                            py[:, 0:512],
                            lhsT=uT[d][:, s * 128:(s + 1) * 128],
                            rhs=wrt_t[d][:, 0:512],
                            start=(d == 0), stop=False)
                        nc.tensor.matmul(
                            py[:, 512:1024],
                            lhsT=uT[d][:, s * 128:(s + 1) * 128],
                            rhs=wrt_t[d][:, 512:1024],
                            start=(d == 0), stop=False)
                    for g in range(RT):
                        nc.tensor.matmul(
                            py[:, g * 128:(g + 1) * 128],
                            lhsT=hr[:, g * CH + s * 128:g * CH + (s + 1) * 128],
                            rhs=ident, is_transpose=True,
                            start=False, stop=True)
                        nc.tensor.matmul(
                            py[:, 512 + g * 128:512 + (g + 1) * 128],
                            lhsT=hi[:, g * CH + s * 128:g * CH + (s + 1) * 128],
                            rhs=ident, is_transpose=True,
                            start=False, stop=True)

                    # layernorm over the 1024 features; sums come free from
                    # ScalarE accum_out and the scalar chain stays on ScalarE
                    # (only the reciprocal needs the DVE)
                    osb = wp.tile([128, 2 * R], F32, name="osb", tag="osb", bufs=1)
                    s2 = sp.tile([128, 1], F32, name="s2", tag="s2")
                    nc.scalar.activation(osb, py, AF.Square, accum_out=s2)
                    s1 = sp.tile([128, 1], F32, name="s1", tag="s1")
                    nc.scalar.activation(osb, py, AF.Copy, accum_out=s1)
                    e2p = sp.tile([128, 1], F32, name="e2p", tag="e2p")
                    nc.scalar.activation(e2p, s2, AF.Identity, bias=epst,
                                         scale=1.0 / (2 * R))
                    mneg = sp.tile([128, 1], F32, name="mneg", tag="mneg")
                    nc.scalar.mul(mneg, s1, -1.0 / (2 * R))
                    msq = sp.tile([128, 1], F32, name="msq", tag="msq")
                    nc.scalar.square(msq, mneg)
                    varep = sp.tile([128, 1], F32, name="varep", tag="varep")
                    nc.scalar.activation(varep, msq, AF.Identity, bias=e2p,
                                         scale=-1.0)
                    rec = sp.tile([128, 1], F32, name="rec", tag="rec")
                    nc.vector.reciprocal(rec, varep)
                    rstd = sp.tile([128, 1], F32, name="rstd", tag="rstd")
                    nc.scalar.sqrt(rstd, rec)
                    nbias = sp.tile([128, 1], F32, name="nbias", tag="nbias")
                    nc.scalar.mul(nbias, mneg, rstd)
                    # py was copied into osb by the mean pass; normalize there
                    # so the PSUM tile frees after two passes
                    nc.scalar.activation(osb, osb, AF.Identity, bias=nbias,
                                         scale=rstd)
                    if apply_gamma_beta:
                        nc.vector.tensor_tensor(osb, osb, gam, op=AX.mult)
                        nc.vector.tensor_tensor(osb, osb, bet, op=AX.add)
                    r0 = c * CH + s * 128
                    nc.sync.dma_start(out=out_b[r0:r0 + 128, :], in_=osb)

                if c == NCH - 1:
                    for g in range(RT):
                        sl = slice(g * CH + CH - 1, g * CH + CH)
                        nc.sync.dma_start(out=fin_r[g * 128:(g + 1) * 128, :],
                                          in_=hr[:, sl])
                        nc.sync.dma_start(out=fin_i[g * 128:(g + 1) * 128, :],
                                          in_=hi[:, sl])

            # software pipeline: PE sees front(i) before back(i-1)
            for c in range(NCH):
                front(c)
                if c > 0:
                    back(c - 1)
            back(NCH - 1)

    nc.compile()
    return nc


def _prep_host(u, h0_r, h0_i, lambda_raw, omega, W_proj, W_res, ln_gamma, ln_beta):
    lam = -5.0 / (1.0 + np.exp(-lambda_raw.astype(np.float64)))
    mag = np.exp(lam).astype(np.float32)

    t_idx = np.arange(1, T + 1, dtype=np.float64)
    ang = omega.astype(np.float64)[:, None] * t_idx[None, :]    # [R, T]
    ctab = np.cos(ang).astype(np.float32)
    stab = np.sin(ang).astype(np.float32)
    # channel-packed [128, RT, T]: row p, group g -> channel g*128+p
    ctab = np.ascontiguousarray(ctab.reshape(RT, 128, T).transpose(1, 0, 2))
    stab = np.ascontiguousarray(stab.reshape(RT, 128, T).transpose(1, 0, 2))
    magb = np.ascontiguousarray(
        np.broadcast_to(mag.reshape(RT, 128).T[:, :, None],
                        (128, RT, CH)).reshape(128, BIG))

    wpt = np.ascontiguousarray(W_proj.T)
    wrt = np.ascontiguousarray(W_res.T)
    idn = np.eye(128, dtype=np.float32)
    gmb = np.ascontiguousarray(
        np.broadcast_to(ln_gamma[None, :], (128, 2 * R))).astype(np.float32)
    btb = np.ascontiguousarray(
        np.broadcast_to(ln_beta[None, :], (128, 2 * R))).astype(np.float32)

    shared = dict(wpt=wpt, wrt=wrt, ctab=ctab, stab=stab, magb=magb, idn=idn,
                  gmb=gmb, btb=btb,
                  epsb=np.full((128, 1), LN_EPS, dtype=np.float32))
    in_maps = []
    for b in range(B):
        m = dict(shared)
        m["u_b"] = np.ascontiguousarray(u[b])
        m["h0r"] = np.ascontiguousarray(h0_r[b][:, None])
        m["h0i"] = np.ascontiguousarray(h0_i[b][:, None])
        in_maps.append(m)
    return in_maps


def kernel(u, h0_r, h0_i, lambda_raw, omega, W_proj, W_res, ln_gamma, ln_beta,
           mm_dtype=None, trace=False):
    mm_dtype = F32R if mm_dtype is None else mm_dtype
    apply_gb = not (np.all(ln_gamma == 1.0) and np.all(ln_beta == 0.0))
    key = (str(mm_dtype), apply_gb)
    if key not in _CACHE:
        _CACHE[key] = build_program(mm_dtype=mm_dtype, apply_gamma_beta=apply_gb)
    nc = _CACHE[key]

    in_maps = _prep_host(u, h0_r, h0_i, lambda_raw, omega, W_proj, W_res,
                         ln_gamma, ln_beta)
    res = run_bass_kernel_spmd(nc, in_maps, core_ids=list(range(B)), trace=trace)

    out = np.stack([res.results[b]["out_b"] for b in range(B)])
    final_r = np.stack([res.results[b]["fin_r"][:, 0] for b in range(B)])
    final_i = np.stack([res.results[b]["fin_i"][:, 0] for b in range(B)])
    kernel.last_results = res
    return out, final_r, final_i


# revision 29
# speedup vs baseline: 1.1379x; 1.1379x over previous
"""Trainium2 Bass kernel for the ComplexSSM problem.

Math (per batch b, channel r):
    lam = -5*sigmoid(lambda_raw); mag = exp(lam); a = mag*exp(i*omega)
    x[t] = W_proj @ u[t]                       (real)
    h[t] = a*h[t-1] + x[t],  h[-1] = h0        (complex, diagonal)
    y[t] = concat(h_r[t], h_i[t]) + W_res @ u[t]
    out  = layernorm(y) * gamma + beta
    finals = h[T-1]

Polar decomposition of the scan (the key trick):
    h[t] = exp(i*omega*(t+1)) * g[t]
    g[t] = mag*g[t-1] + exp(-i*omega*(t+1))*x[t],   g[-1] = h0
so with C[t]=cos(omega*(t+1)), S[t]=sin(omega*(t+1)) (host fp64 tables):
    g_r = scan(mag, x*C, op1=add),  g_i = scan(mag, x*S, op1=subtract)
    h_r = C*g_r - S*g_i,            h_i = C*g_i + S*g_r
Each scan is a real first-order recurrence with constant per-partition
coefficient -> native tensor_tensor_scan on the DVE.

Device layout per core (one batch per NeuronCore, 8 cores):
    - time processed in chunks of CH=512, software-pipelined so the PE
      stream of chunk i's front end (u transpose + x matmuls) comes before
      chunk i-1's back end (residual matmuls + h transposes + layernorm),
      keeping the PE dense while the scan chain runs on DVE/GPSIMD.
    - channel dim (R=512) packed as 4 partition-tiles side by side in the
      free dim: "big tiles" [128, 4*CH] so elementwise work runs in few
      large instructions.
    - rotation products are computed in place to save SBUF.
"""

import sys

if "/opt/trn_rl_repo" not in sys.path:
    sys.path.insert(0, "/opt/trn_rl_repo")

import numpy as np

import concourse.bacc as bacc
import concourse.mybir as mybir
import concourse.tile as tile
from concourse.bass_utils import run_bass_kernel_spmd

# Problem constants (hardcoded per the contract)
B, T, D, R = 8, 4096, 1024, 512
CH = 512          # time chunk (scan segment length)
NCH = T // CH     # 8 chunks
RT = R // 128     # 4 channel tiles
DT = D // 128     # 8 contraction tiles
SUB = CH // 128   # 4 psum output subtiles per chunk
BIG = RT * CH     # packed free size of channel-major big tiles
LN_EPS = 1e-5

F32 = mybir.dt.float32
F32R = mybir.dt.float32r
AX = mybir.AluOpType
AF = mybir.ActivationFunctionType

_CACHE = {}


def build_program(mm_dtype=F32, apply_gamma_beta=False):
    nc = bacc.Bacc("TRN2", target_bir_lowering=False, debug=False, num_devices=B)

    u_b = nc.dram_tensor("u_b", [T, D], F32, kind="ExternalInput").ap()
    h0r = nc.dram_tensor("h0r", [R, 1], F32, kind="ExternalInput").ap()
    h0i = nc.dram_tensor("h0i", [R, 1], F32, kind="ExternalInput").ap()
    wpt = nc.dram_tensor("wpt", [D, R], F32, kind="ExternalInput").ap()      # W_proj.T
    wrt = nc.dram_tensor("wrt", [D, 2 * R], F32, kind="ExternalInput").ap()  # W_res.T
    # channel-packed rotation tables: [128, RT, T]
    ctab = nc.dram_tensor("ctab", [128, RT, T], F32, kind="ExternalInput").ap()
    stab = nc.dram_tensor("stab", [128, RT, T], F32, kind="ExternalInput").ap()
    magb = nc.dram_tensor("magb", [128, BIG], F32, kind="ExternalInput").ap()
    idn = nc.dram_tensor("idn", [128, 128], F32, kind="ExternalInput").ap()
    epsb = nc.dram_tensor("epsb", [128, 1], F32, kind="ExternalInput").ap()
    gmb = nc.dram_tensor("gmb", [128, 2 * R], F32, kind="ExternalInput").ap()
    btb = nc.dram_tensor("btb", [128, 2 * R], F32, kind="ExternalInput").ap()

    out_b = nc.dram_tensor("out_b", [T, 2 * R], F32, kind="ExternalOutput").ap()
    fin_r = nc.dram_tensor("fin_r", [R, 1], F32, kind="ExternalOutput").ap()
    fin_i = nc.dram_tensor("fin_i", [R, 1], F32, kind="ExternalOutput").ap()

    WDT = mm_dtype

    with tile.TileContext(nc) as tc:
        with tc.tile_pool(name="const", bufs=1) as cp, \
             tc.tile_pool(name="work", bufs=1) as wp, \
             tc.tile_pool(name="small", bufs=2) as sp, \
             tc.tile_pool(name="pt", bufs=2, space="PSUM") as ptp, \
             tc.tile_pool(name="px", bufs=2, space="PSUM") as pxp, \
             tc.tile_pool(name="py", bufs=2, space="PSUM") as pyp:

            # ---- static loads ----
            ident = cp.tile([128, 128], F32, name="ident", tag="ident")
            nc.sync.dma_start(out=ident, in_=idn)
            epst = cp.tile([128, 1], F32, name="epst", tag="epst")
            nc.sync.dma_start(out=epst, in_=epsb)
            wpt_t = []
            wrt_t = []
            for d in range(DT):
                w1 = cp.tile([128, R], WDT, name=f"wpt{d}", tag=f"wpt{d}")
                w2 = cp.tile([128, 2 * R], WDT, name=f"wrt{d}", tag=f"wrt{d}")
                nc.sync.dma_start(out=w1,
                                  in_=wpt[d * 128:(d + 1) * 128, :].bitcast(WDT))
                nc.sync.dma_start(out=w2,
                                  in_=wrt[d * 128:(d + 1) * 128, :].bitcast(WDT))
                wpt_t.append(w1)
                wrt_t.append(w2)
            mag_t = cp.tile([128, BIG], F32, name="mag_t", tag="mag_t")
            nc.sync.dma_start(out=mag_t, in_=magb)
            h0r_t = []
            h0i_t = []
            for g in range(RT):
                hr0 = cp.tile([128, 1], F32, name=f"h0r{g}", tag=f"h0r{g}")
                nc.sync.dma_start(out=hr0, in_=h0r[g * 128:(g + 1) * 128, :])
                h0r_t.append(hr0)
                hi0 = cp.tile([128, 1], F32, name=f"h0i{g}", tag=f"h0i{g}")
                nc.sync.dma_start(out=hi0, in_=h0i[g * 128:(g + 1) * 128, :])
                h0i_t.append(hi0)
            if apply_gamma_beta:
                gam = cp.tile([128, 2 * R], F32, name="gam", tag="gam")
                nc.sync.dma_start(out=gam, in_=gmb)
                bet = cp.tile([128, 2 * R], F32, name="bet", tag="bet")
                nc.sync.dma_start(out=bet, in_=btb)

            carry_r = None
            carry_i = None
            state = {}

            def front(c):
                """u load/transpose, x matmuls, rotations + scans for chunk c."""
                nonlocal carry_r, carry_i
                uraw = []
                for s in range(SUB):
                    ur = wp.tile([128, D], F32, name=f"uraw{s}", tag=f"uraw{s}",
                                 bufs=1)
                    r0 = c * CH + s * 128
                    nc.gpsimd.dma_start(out=ur, in_=u_b[r0:r0 + 128, :])
                    uraw.append(ur)

                uT = []
                for d in range(DT):
                    pt = ptp.tile([128, CH], F32, name=f"pt{d}", tag="pt")
                    for s in range(SUB):
                        nc.tensor.matmul(
                            pt[:, s * 128:(s + 1) * 128],
                            lhsT=uraw[s][:, d * 128:(d + 1) * 128],
                            rhs=ident, is_transpose=True,
                            start=True, stop=True)
                    ut = wp.tile([128, CH], WDT, name=f"uT{d}", tag=f"uT{d}",
                                 bufs=2)
                    nc.scalar.copy(ut, pt)
                    uT.append(ut)

                ct = wp.tile([128, BIG], F32, name="ct", tag="ct", bufs=2)
                nc.sync.dma_start(
                    out=ct.rearrange("p (g t) -> p g t", g=RT),
                    in_=ctab[:, :, c * CH:(c + 1) * CH])
                st = wp.tile([128, BIG], F32, name="st", tag="st", bufs=2)
                nc.sync.dma_start(
                    out=st.rearrange("p (g t) -> p g t", g=RT),
                    in_=stab[:, :, c * CH:(c + 1) * CH])

                # x projection -> xsb (via ScalarE copy off PSUM)
                xsb = wp.tile([128, BIG], F32, name="xsb", tag="xsb", bufs=2)
                for g in range(RT):
                    px = pxp.tile([128, CH], F32, name=f"px{g}", tag="px")
                    for d in range(DT):
                        nc.tensor.matmul(
                            px,
                            lhsT=wpt_t[d][:, g * 128:(g + 1) * 128],
                            rhs=uT[d],
                            start=(d == 0), stop=(d == DT - 1))
                    nc.scalar.copy(xsb[:, g * CH:(g + 1) * CH], px)

                # pre-rotation (DVE): xtr = x*C, xti = x*S
                xtr = wp.tile([128, BIG], F32, name="xtr", tag="xa", bufs=1)
                nc.vector.tensor_tensor(xtr, xsb, ct, op=AX.mult)
                xti = wp.tile([128, BIG], F32, name="xti", tag="xb", bufs=1)
                nc.vector.tensor_tensor(xti, xsb, st, op=AX.mult)

                # scans (DVE) per channel-tile segment; carries per segment
                gr = wp.tile([128, BIG], F32, name="gr", tag="gA", bufs=2)
                gi = wp.tile([128, BIG], F32, name="gi", tag="gB", bufs=2)
                ncar_r = []
                ncar_i = []
                for g in range(RT):
                    sl = slice(g * CH, (g + 1) * CH)
                    init_r = h0r_t[g] if c == 0 else carry_r[g]
                    nc.vector.tensor_tensor_scan(
                        gr[:, sl], mag_t[:, sl], xtr[:, sl], init_r,
                        op0=AX.mult, op1=AX.add)
                    init_i = h0i_t[g] if c == 0 else carry_i[g]
                    nc.vector.tensor_tensor_scan(
                        gi[:, sl], mag_t[:, sl], xti[:, sl], init_i,
                        op0=AX.mult, op1=AX.subtract)
                    ncr = sp.tile([128, 1], F32, name=f"ncr{g}", tag=f"ncr{g}",
                                  bufs=2)
                    nc.vector.tensor_copy(ncr, gr[:, (g + 1) * CH - 1:(g + 1) * CH])
                    ncar_r.append(ncr)
                    nci = sp.tile([128, 1], F32, name=f"nci{g}", tag=f"nci{g}",
                                  bufs=2)
                    nc.vector.tensor_copy(nci, gi[:, (g + 1) * CH - 1:(g + 1) * CH])
                    ncar_i.append(nci)
                carry_r, carry_i = ncar_r, ncar_i

                # post-rotation (all DVE; GPSIMD is avoided entirely since it
                # contends for the DVE SBUF port):
                #   hr = C*gr - S*gi ; hi = C*gi + S*gr
                ta = wp.tile([128, BIG], F32, name="ta", tag="xa", bufs=1)
                nc.vector.tensor_tensor(ta, ct, gr, op=AX.mult)
                tb = wp.tile([128, BIG], F32, name="tb", tag="xb", bufs=1)
                nc.vector.tensor_tensor(tb, st, gi, op=AX.mult)
                hr = wp.tile([128, BIG], F32, name="hr", tag="gA", bufs=2)
                nc.vector.tensor_tensor(hr, ta, tb, op=AX.subtract)
                tc_ = wp.tile([128, BIG], F32, name="tc_", tag="xa", bufs=1)
                nc.vector.tensor_tensor(tc_, ct, gi, op=AX.mult)
                td = wp.tile([128, BIG], F32, name="td", tag="xb", bufs=1)
                nc.vector.tensor_tensor(td, st, gr, op=AX.mult)
                hi = wp.tile([128, BIG], F32, name="hi", tag="gB", bufs=2)
                nc.vector.tensor_tensor(hi, tc_, td, op=AX.add)
                state[c] = (uT, hr, hi)

            def back(c):
                """residual matmuls + h transposes + layernorm for chunk c."""
                uT, hr, hi = state.pop(c)
                for s in range(SUB):
                    py = pyp.tile([128, 2 * R], F32, name="py", tag="py")
                    for d in range(DT):
                        nc.tensor.matmul(
                            py[:, 0:512],
                            lhsT=uT[d][:, s * 128:(s + 1) * 128],
                            rhs=wrt_t[d][:, 0:512],
                            start=(d == 0), stop=False)
                        nc.tensor.matmul(
                            py[:, 512:1024],
                            lhsT=uT[d][:, s * 128:(s + 1) * 128],
                            rhs=wrt_t[d][:, 512:1024],
                            start=(d == 0), stop=False)
                    for g in range(RT):
                        nc.tensor.matmul(
                            py[:, g * 128:(g + 1) * 128],
                            lhsT=hr[:, g * CH + s * 128:g * CH + (s + 1) * 128],
                            rhs=ident, is_transpose=True,
                            start=False, stop=True)
                        nc.tensor.matmul(
                            py[:, 512 + g * 128:512 + (g + 1) * 128],
                            lhsT=hi[:, g * CH + s * 128:g * CH + (s + 1) * 128],
                            rhs=ident, is_transpose=True,
                            start=False, stop=True)

                    # layernorm over the 1024 features; sums come free from
                    # ScalarE accum_out and the scalar chain stays on ScalarE
                    # (only the reciprocal needs the DVE)
                    osb = wp.tile([128, 2 * R], F32, name="osb", tag="osb", bufs=1)
                    s2 = sp.tile([128, 1], F32, name="s2", tag="s2")
                    nc.scalar.activation(osb, py, AF.Square, accum_out=s2)
                    s1 = sp.tile([128, 1], F32, name="s1", tag="s1")
                    nc.scalar.activation(osb, py, AF.Copy, accum_out=s1)
                    e2p = sp.tile([128, 1], F32, name="e2p", tag="e2p")
                    nc.scalar.activation(e2p, s2, AF.Identity, bias=epst,
                                         scale=1.0 / (2 * R))
                    mneg = sp.tile([128, 1], F32, name="mneg", tag="mneg")
                    nc.scalar.mul(mneg, s1, -1.0 / (2 * R))
                    msq = sp.tile([128, 1], F32, name="msq", tag="msq")
                    nc.scalar.square(msq, mneg)
                    varep = sp.tile([128, 1], F32, name="varep", tag="varep")
                    nc.scalar.activation(varep, msq, AF.Identity, bias=e2p,
                                         scale=-1.0)
                    rec = sp.tile([128, 1], F32, name="rec", tag="rec")
                    nc.vector.reciprocal(rec, varep)
                    rstd = sp.tile([128, 1], F32, name="rstd", tag="rstd")
                    nc.scalar.sqrt(rstd, rec)
                    nbias = sp.tile([128, 1], F32, name="nbias", tag="nbias")
                    nc.scalar.mul(nbias, mneg, rstd)
                    # py was copied into osb by the mean pass; normalize there
                    # so the PSUM tile frees after two passes
                    nc.scalar.activation(osb, osb, AF.Identity, bias=nbias,
                                         scale=rstd)
                    if apply_gamma_beta:
                        nc.vector.tensor_tensor(osb, osb, gam, op=AX.mult)
                        nc.vector.tensor_tensor(osb, osb, bet, op=AX.add)
                    r0 = c * CH + s * 128
                    nc.sync.dma_start(out=out_b[r0:r0 + 128, :], in_=osb)

                if c == NCH - 1:
                    for g in range(RT):
                        sl = slice(g * CH + CH - 1, g * CH + CH)
                        nc.sync.dma_start(out=fin_r[g * 128:(g + 1) * 128, :],
                                          in_=hr[:, sl])
                        nc.sync.dma_start(out=fin_i[g * 128:(g + 1) * 128, :],
                                          in_=hi[:, sl])

            # software pipeline: PE sees front(i) before back(i-1)
            for c in range(NCH):
                front(c)
                if c > 0:
                    back(c - 1)
            back(NCH - 1)

    nc.compile()
    return nc


def _prep_host(u, h0_r, h0_i, lambda_raw, omega, W_proj, W_res, ln_gamma, ln_beta):
    lam = -5.0 / (1.0 + np.exp(-lambda_raw.astype(np.float64)))
    mag = np.exp(lam).astype(np.float32)

    t_idx = np.arange(1, T + 1, dtype=np.float64)
    ang = omega.astype(np.float64)[:, None] * t_idx[None, :]    # [R, T]
    ctab = np.cos(ang).astype(np.float32)
    stab = np.sin(ang).astype(np.float32)
    # channel-packed [128, RT, T]: row p, group g -> channel g*128+p
    ctab = np.ascontiguousarray(ctab.reshape(RT, 128, T).transpose(1, 0, 2))
    stab = np.ascontiguousarray(stab.reshape(RT, 128, T).transpose(1, 0, 2))
    magb = np.ascontiguousarray(
        np.broadcast_to(mag.reshape(RT, 128).T[:, :, None],
                        (128, RT, CH)).reshape(128, BIG))

    wpt = np.ascontiguousarray(W_proj.T)
    wrt = np.ascontiguousarray(W_res.T)
    idn = np.eye(128, dtype=np.float32)
    gmb = np.ascontiguousarray(
        np.broadcast_to(ln_gamma[None, :], (128, 2 * R))).astype(np.float32)
    btb = np.ascontiguousarray(
        np.broadcast_to(ln_beta[None, :], (128, 2 * R))).astype(np.float32)

    shared = dict(wpt=wpt, wrt=wrt, ctab=ctab, stab=stab, magb=magb, idn=idn,
                  gmb=gmb, btb=btb,
                  epsb=np.full((128, 1), LN_EPS, dtype=np.float32))
    in_maps = []
    for b in range(B):
        m = dict(shared)
        m["u_b"] = np.ascontiguousarray(u[b])
        m["h0r"] = np.ascontiguousarray(h0_r[b][:, None])
        m["h0i"] = np.ascontiguousarray(h0_i[b][:, None])
        in_maps.append(m)
    return in_maps


def kernel(u, h0_r, h0_i, lambda_raw, omega, W_proj, W_res, ln_gamma, ln_beta,
           mm_dtype=None, trace=False):
    mm_dtype = F32R if mm_dtype is None else mm_dtype
    apply_gb = not (np.all(ln_gamma == 1.0) and np.all(ln_beta == 0.0))
    key = (str(mm_dtype), apply_gb)
    if key not in _CACHE:
        _CACHE[key] = build_program(mm_dtype=mm_dtype, apply_gamma_beta=apply_gb)
    nc = _CACHE[key]

    in_maps = _prep_host(u, h0_r, h0_i, lambda_raw, omega, W_proj, W_res,
                         ln_gamma, ln_beta)
    res = run_bass_kernel_spmd(nc, in_maps, core_ids=list(range(B)), trace=trace)

    out = np.stack([res.results[b]["out_b"] for b in range(B)])
    final_r = np.stack([res.results[b]["fin_r"][:, 0] for b in range(B)])
    final_i = np.stack([res.results[b]["fin_i"][:, 0] for b in range(B)])
    kernel.last_results = res
    return out, final_r, final_i
